# revision 48
# baseline (speedup 1.0000x reference)
"""Multi-head attention (B=4, T=2048, D=1024, H=16) on 8 Trainium2 NeuronCores.

Sharding: core = (batch, head-group): b = core // 2, g = core % 2.
Each core computes heads [g*8, g*8+8) of batch b:
  - Q/K projections into transposed layout qT/kT = W_g @ x_b.T  [512, 2048]
  - V projection in natural layout [2048, 512] plus a ones column per head
  - scores computed transposed: S.T tile = K_h @ Q_h.T on the PE; exp fused
    on ScalarE over two-bank PSUM groups (FD=1024), scale=1/sqrt(64),
    no max subtraction (logits ~N(0,1))
  - PV transposed: out[128q, 65] = pT-slice[128k,128q].T @ [V_h|1][128k,65]
    so the matmul streams only 65 columns per k-tile (the PE charges
    N=out-free-size) and the softmax row-sum lands lane-wise in column 64
  - normalize is a per-partition reciprocal + tensor_scalar_mul on DVE
  - o is transposed back for the output projection: XBAR dma transpose when
    deferred (rows 0-2), PE transpose-matmul on the latency-critical last row
  - partial output projection yT_g = Wo[:, g].T-contraction  [1024, 2048] bf16
Host: y[b] = (yT_part[2b] + yT_part[2b+1]).T + bo + bv @ Wo.T
(softmax rows sum to one, so the V bias contributes exactly bv @ Wo.T).

Schedule: pair-row outer, query-chunk inner; 16 windows of 16 score/exp
groups each. ScalarE's exp stream (256 x ~1.04us) is the near-critical
engine, so every other PE obligation — the trailing PV of the previous
window, Q/K prefetch for the next pair-row, the V projection (spread one
head-pair per row across the row boundary), and the output projection of
the previous chunk — is emitted as generator "filler" steps pulled between
score groups at ~600 ns of filler per group, keeping both PE and ScalarE
continuously busy. Input DMAs are few and big (shared-HWDGE cost), ordered
exactly as the prologue consumes them, with mt-major weight layouts so the
first projection slices are single fat descriptors.

Self-contained: hardcodes all shapes; requires only concourse (bass) + numpy.
"""

import numpy as np

B, T, D = 4, 2048, 1024
H, HD = 16, 64
HG, DG = 8, 512          # heads / feature columns per core
NCORES = 8
P = 128
KD = D // P              # 8  k-tiles over model dim
MQ = DG // P             # 4  partition tiles of qT/kT/oT (one per head pair)
TK = T // P              # 16 key tiles
TQC = 512                # query-chunk (= one fp32 PSUM bank)
NC2 = T // TQC           # 4  query chunks
VW = HD + 1              # V columns per head incl. ones column
SCALE = 0.125            # 1/sqrt(HD)
PULL_RATE = 600.0        # ns of filler pulled per score group
DMA_TP = True            # o-transpose on the DMA XBAR instead of the PE

_CACHE: dict = {}


def _emit(tc, aps, reps=1):
    import concourse.bass as bass  # noqa: F401
    from concourse import masks, mybir

    nc = tc.nc
    dt = mybir.dt
    f32, bf16 = dt.float32, dt.bfloat16
    AF = mybir.ActivationFunctionType
    xT, wq, wk, wv, wo, bq, bk, yT = (
        aps["xT"], aps["wq"], aps["wk"], aps["wv"], aps["wo"],
        aps["bq"], aps["bk"], aps["yT"],
    )

    from contextlib import ExitStack

    with ExitStack() as ctx:
        const = ctx.enter_context(tc.tile_pool(name="const", bufs=1))
        persist = ctx.enter_context(tc.tile_pool(name="persist", bufs=1))
        xw = ctx.enter_context(tc.tile_pool(name="xw", bufs=1))
        ptp = ctx.enter_context(tc.tile_pool(name="ptp", bufs=4))
        onp = ctx.enter_context(tc.tile_pool(name="onp", bufs=3))
        yop = ctx.enter_context(tc.tile_pool(name="yop", bufs=3))
        nrm = ctx.enter_context(tc.tile_pool(name="nrm", bufs=3))
        scps = ctx.enter_context(tc.tile_pool(name="scps", bufs=2, space="PSUM"))
        qkvps = ctx.enter_context(tc.tile_pool(name="qkvps", bufs=2, space="PSUM"))
        pvtp = ctx.enter_context(tc.tile_pool(name="pvtp", bufs=2, space="PSUM"))

        # ---- persistent SBUF ----
        q_sb = persist.tile([P, MQ, T], bf16)
        k_sb = persist.tile([P, MQ, T], bf16)
        v_sb = persist.tile([P, TK, HG * VW], bf16)
        oT_sb = persist.tile([P, MQ, T], bf16)
        v4d = v_sb.rearrange("p t (h c) -> p t h c", h=HG)
        nc.vector.memset(v4d[:, :, :, HD : HD + 1], 1.0)
        ident = const.tile([P, P], bf16)
        masks.make_identity(nc, ident)

        # ---- input DMAs. The shared HWDGE device costs ~625 ns per DMA, so
        # transfers are few and big, emitted on one queue in exactly the
        # order the prologue consumes them: the mt0 slices of wk/wq and x
        # chunk 0 arrive first (split in ki halves so the first projection
        # groups overlap the transfer), then the later x chunks, then the
        # weights the fillers need later.
        x_sb = xw.tile([P, KD, T], bf16)
        wq_sb = xw.tile([P, MQ, KD, P], bf16)
        wk_sb = xw.tile([P, MQ, KD, P], bf16)
        bq_sb = const.tile([P, MQ], f32)
        bk_sb = const.tile([P, MQ], f32)
        wv_sb = xw.tile([P, KD, DG], bf16)
        wo_sb = const.tile([P, MQ, D], bf16)
        nc.sync.dma_start(out=wk_sb[:, 0], in_=wk[:, 0])
        nc.sync.dma_start(out=x_sb[:, :, 0:TQC], in_=xT[:, :, 0:TQC])
        nc.sync.dma_start(out=wq_sb[:, 0], in_=wq[:, 0])
        nc.sync.dma_start(out=bq_sb, in_=bq)
        nc.sync.dma_start(out=bk_sb, in_=bk)
        nc.sync.dma_start(out=wv_sb, in_=wv)
        for xc in (1, 2, 3):
            s = slice(xc * TQC, (xc + 1) * TQC)
            nc.sync.dma_start(out=x_sb[:, :, s], in_=xT[:, :, s])
        nc.sync.dma_start(out=wk_sb[:, 1:MQ], in_=wk[:, 1:MQ])
        nc.sync.dma_start(out=wq_sb[:, 1:MQ], in_=wq[:, 1:MQ])
        nc.sync.dma_start(out=wo_sb, in_=wo)

        # accumulation order matches the ki-half arrival of the first pieces
        ki_order = list(range(KD))

        def gen_qk(mt, n, which="kq"):
            """Generator: one T-chunk (n) of the q and/or k projection."""
            sel = {
                "k": ((wk_sb, bk_sb, k_sb),),
                "q": ((wq_sb, bq_sb, q_sb),),
                "kq": ((wk_sb, bk_sb, k_sb), (wq_sb, bq_sb, q_sb)),
            }[which]
            for w_sb, b_col, dst in sel:
                ps = qkvps.tile([P, TQC], f32, tag="qkv", name="ps_qkv")
                for idx, ki in enumerate(ki_order):
                    nc.tensor.matmul(
                        ps,
                        w_sb[:, mt, ki, :],
                        x_sb[:, ki, n * TQC : (n + 1) * TQC],
                        start=(idx == 0),
                        stop=(idx == KD - 1),
                    )
                    if idx % 2 == 1 and idx < KD - 1:
                        yield 427
                nc.vector.tensor_scalar_add(
                    dst[:, mt, n * TQC : (n + 1) * TQC], ps, b_col[:, mt : mt + 1]
                )
                yield 427

        def gen_v(vp, t0, t1):
            """Generator: V projection of head pair vp for token tiles
            [t0, t1) — split by pair so it spreads across rows 0-2."""
            for t in range(t0, t1):
                ps = qkvps.tile([P, P], f32, tag="qkv", name="ps_v")
                for idx, ki in enumerate(ki_order):
                    nc.tensor.matmul(
                        ps,
                        x_sb[:, ki, t * P : (t + 1) * P],
                        wv_sb[:, ki, vp * P : (vp + 1) * P],
                        start=(idx == 0),
                        stop=(idx == KD - 1),
                    )
                    if idx == 3:
                        yield 213
                nc.vector.tensor_copy(
                    v4d[:, t, 2 * vp : 2 * vp + 2, 0:HD],
                    ps.rearrange("p (h c) -> p h c", h=2),
                )
                yield 213

        def gen_pvt(p, tq0, qw, pts, holder=None):
            """Two generators (head 0, head 1) of transposed PV + normalize
            (+ o-transpose unless deferred via holder) for the window at
            (p, tq0, qw). Head 1 depends on the last exps of that window's
            stream, so its generator is scheduled at the end of the next
            window's fillers."""
            nqt = qw // P
            ntile = (nqt + 1) // 2
            st = {}

            def part(i):
                if i == 0:
                    st["o_nat"] = onp.tile([P, nqt, P], bf16, name="onat")
                    if holder is not None:
                        holder["o_nat"] = st["o_nat"]
                    st["pv"] = [
                        pvtp.tile([P, 2, 2, VW], f32, tag="pv", name=f"pv{t}")
                        for t in range(ntile)
                    ]
                o_nat, pv = st["o_nat"], st["pv"]
                h = 2 * p + i
                # a matmul start lazily zeroes its whole 2KB psum bank, so
                # each (t, j, i) accumulation group runs start-to-stop
                # before the next opens
                for t in range(ntile):
                    for j in range(2):
                        qs = (2 * t + j) * P
                        for half in range(2):
                            for tk in range(8 * half, 8 * half + 8):
                                nc.tensor.matmul(
                                    pv[t][:, j, i, :],
                                    pts[i][:, tk, qs : qs + P],
                                    v_sb[:, tk, h * VW : (h + 1) * VW],
                                    start=(tk == 0),
                                    stop=(tk == TK - 1),
                                )
                            yield 217
                if i == 0:
                    return
                for t in range(ntile):
                    rc = nrm.tile([P, 2, 2, 1], f32, name="rc")
                    nc.vector.reciprocal(rc, pv[t][:, :, :, HD : HD + 1])
                    for j in range(2):
                        for ii in range(2):
                            nc.vector.tensor_scalar_mul(
                                o_nat[:, 2 * t + j, ii * HD : (ii + 1) * HD],
                                pv[t][:, j, ii, 0:HD],
                                rc[:, j, ii, :],
                            )
                    yield 0
                if holder is None:
                    yield from gen_tp(p, tq0, qw, o_nat)

            return part(0), part(1)

        def gen_tp(p, tq0, qw, o_nat, dma=False):
            if isinstance(o_nat, dict):
                o_nat = o_nat["o_nat"]
            nqt = qw // P
            if DMA_TP and dma:
                # XBAR transpose: out[dg, qt, q] = o_nat[q, qt, dg]; no PE or
                # DVE time, just a DMA queue slot
                nc.sync.dma_start_transpose(
                    out=oT_sb[:, p, tq0 : tq0 + qw].rearrange(
                        "p (a b) -> p a b", a=nqt
                    ),
                    in_=o_nat[:, :, :],
                )
                yield 213
                return
            tpv = pvtp.tile([P, nqt, P], bf16, tag="pv", name="tpv")
            for qt in range(nqt):
                nc.tensor.transpose(tpv[:, qt, :], o_nat[:, qt, :], ident)
            nc.vector.tensor_copy(
                oT_sb[:, p, tq0 : tq0 + qw], tpv.rearrange("p a b -> p (a b)")
            )
            yield 213

        def gen_oproj(tq0, qw):
            for j in range(D // P):
                ys = qkvps.tile([P, qw], f32, tag="qkv", name="ys")
                for ki in range(MQ):
                    nc.tensor.matmul(
                        ys,
                        wo_sb[:, ki, j * P : (j + 1) * P],
                        oT_sb[:, ki, tq0 : tq0 + qw],
                        start=(ki == 0),
                        stop=(ki == MQ - 1),
                    )
                yo = yop.tile([P, qw], bf16, name="yo")
                nc.vector.tensor_copy(yo, ys)
                nc.sync.dma_start(out=yT[:, j, tq0 : tq0 + qw], in_=yo)
                yield 853 * qw // TQC

        def gen_spacer(ns):
            yield ns

        def sc_group(pt, p, hb, tq0, qw, g):
            scs = scps.tile([P, 2, qw], f32, tag="sc", name="sc")
            for u in range(2):
                tk = 2 * g + u
                nc.tensor.matmul(
                    scs[:, u, :],
                    k_sb[hb : hb + HD, p, tk * P : (tk + 1) * P],
                    q_sb[hb : hb + HD, p, tq0 : tq0 + qw],
                    start=True,
                    stop=True,
                )
            nc.scalar.activation(pt[:, 2 * g : 2 * g + 2, :], scs, AF.Exp, scale=SCALE)

        def drain(gens):
            while gens:
                try:
                    return next(gens[0])
                except StopIteration:
                    gens.pop(0)
            return None

        def window(p, tq0, qw, pts, gens, interleave_heads=False, skip=0):
            # head-major keeps each head's exps contiguous so the trailing PV
            # (head-outer) never waits; window (0,0) instead paces key-tiles
            # slowest (head-interleaved) so the k(mt0, n) fillers stay ahead
            # of the score groups that read them
            if interleave_heads:
                order = [(g // 2, g % 2) for g in range(16)]
            else:
                order = [(g % 8, g // 8) for g in range(16)][skip:]
            rate = PULL_RATE * qw / TQC
            debt = 0.0
            for g, i in order:
                sc_group(pts[i], p, i * HD, tq0, qw, g)
                debt += rate
                while debt > 0:
                    step = drain(gens)
                    if step is None:
                        debt = 0.0
                        break
                    debt -= max(step, 180)
            while drain(gens) is not None:
                pass

        # ---- schedule: pair-outer, chunk-inner; PV trails one window;
        # V-projection of pair p+1 spreads across row p to even out the
        # per-window PE load against ScalarE's fixed 16-exp stream ----
        if reps > 1:
            loop_cm = tc.For_i(0, reps, 1)
            loop_cm.__enter__()

        for g_ in gen_qk(0, 0, "k"):
            pass
        for g_ in gen_qk(0, 0, "q"):
            pass

        prev = None
        tp_pending = []  # deferred o-transposes for rows 0-2
        full = [(n * TQC, TQC) for n in range(NC2)]
        last_row = full
        pts_next, banked = None, 0
        for p in range(MQ):
            chunks = last_row if p == MQ - 1 else full
            for c, (tq0, qw) in enumerate(chunks):
                if pts_next is not None:
                    pts, skip = pts_next, banked
                    pts_next, banked = None, 0
                else:
                    skip = 0
                    pts = [
                        ptp.tile([P, TK, qw], bf16, tag="pt", name="pt0"),
                        ptp.tile([P, TK, qw], bf16, tag="pt", name="pt1"),
                    ]
                gens = []
                if p == 0 and c == 0:
                    # k(mt0, n) fillers interleave with short V slices; the
                    # pull budget completes chunk n before the score groups
                    # that read it (emission-order margins checked)
                    gens += [gen_qk(0, 1, "k"), gen_v(0, 0, 2),
                             gen_qk(0, 2, "k"), gen_v(0, 2, 4),
                             gen_qk(0, 3, "k"), gen_v(0, 4, TK)]
                else:
                    # V of pair p+1 spreads over windows (p,1)..(p+1,0): the
                    # row-boundary windows are otherwise the lightest
                    vp, vt = (p + 1, 4 * (c - 1)) if c else (p, 12)
                    if vp <= MQ - 1:
                        gens.append(gen_v(vp, vt, vt + 4))
                if len(tp_pending) > 1 or (tp_pending and p == MQ - 1):
                    gens.append(gen_tp(*tp_pending.pop(0), dma=True))
                pvt1 = None
                if prev is not None:
                    pp, ptq0, pqw, ppts = prev
                    if pp == MQ - 1:
                        pvt0, pvt1 = gen_pvt(pp, ptq0, pqw, ppts)
                    else:
                        holder = {}
                        pvt0, pvt1 = gen_pvt(pp, ptq0, pqw, ppts, holder)
                        tp_pending.append((pp, ptq0, pqw, holder))
                    gens.append(pvt0)
                    gens.append(pvt1)
                    pvt1 = None
                if p == 0 and c < NC2 - 1:
                    gens.append(gen_qk(0, c + 1, "q"))
                if p < MQ - 1:
                    which = "k" if (p == MQ - 2 and c == NC2 - 1) else "kq"
                    gens.append(gen_qk(p + 1, c, which))
                if p == MQ - 1 and c == 0:
                    gens.append(gen_qk(MQ - 1, NC2 - 1, "q"))
                if pvt1 is not None:
                    gens.append(pvt1)
                if p == MQ - 1 and c > 0:
                    gens.append(gen_spacer(1200))
                    gens.append(gen_oproj(*chunks[c - 1]))
                window(p, tq0, qw, pts, gens,
                       interleave_heads=(p == 0 and c == 0), skip=skip)
                prev = (p, tq0, qw, pts)

        # epilogue: leftover transposes, last window's PV + final projection
        gens = [gen_tp(*t, dma=True) for t in tp_pending]
        gens += [*gen_pvt(*prev), gen_spacer(600), gen_oproj(*last_row[-1])]
        while drain(gens) is not None:
            pass

        if reps > 1:
            loop_cm.__exit__(None, None, None)


def _build(debug=False, reps=1):
    import concourse.tile as tile
    from concourse import bacc, mybir

    dt = mybir.dt
    f32, bf16 = dt.float32, dt.bfloat16

    nc = bacc.Bacc("TRN2", target_bir_lowering=False, debug=False)
    # inputs are host-preswizzled into partition-major layouts so every DMA
    # descriptor is a fat contiguous run
    aps = {
        "xT": nc.dram_tensor("xT", [P, KD, T], bf16, kind="ExternalInput").ap(),
        "wq": nc.dram_tensor(
            "wq", [P, MQ, KD, P], bf16, kind="ExternalInput"
        ).ap(),
        "wk": nc.dram_tensor(
            "wk", [P, MQ, KD, P], bf16, kind="ExternalInput"
        ).ap(),
        "wv": nc.dram_tensor("wv", [P, KD, DG], bf16, kind="ExternalInput").ap(),
        "wo": nc.dram_tensor("wo", [P, MQ, D], bf16, kind="ExternalInput").ap(),
        "bq": nc.dram_tensor("bq", [P, MQ], f32, kind="ExternalInput").ap(),
        "bk": nc.dram_tensor("bk", [P, MQ], f32, kind="ExternalInput").ap(),
        "yT": nc.dram_tensor("yT", [P, D // P, T], bf16, kind="ExternalOutput").ap(),
    }

    with tile.TileContext(nc) as tc:
        _emit(tc, aps, reps=reps)
    nc.compile()
    return nc


def _get_nc():
    if "nc" not in _CACHE:
        _CACHE["nc"] = _build()
    return _CACHE["nc"]


def _shard_inputs(x, Wq, bq, Wk, bk, Wv, bv, Wo, bo):
    import ml_dtypes

    bf16 = ml_dtypes.bfloat16
    f32 = np.float32

    def c(a, dtype):
        return np.ascontiguousarray(a).astype(dtype)

    def kp(a, kt):  # [kt*P, F] -> [P, kt, F] partition-major swizzle
        return a.reshape(kt, P, a.shape[-1]).transpose(1, 0, 2)

    in_maps = []
    for core in range(NCORES):
        b, g = core // 2, core % 2
        hs = g * DG
        in_maps.append(
            {
                "xT": c(kp(x[b].T, KD), bf16),
                "wq": c(
                    kp(Wq[hs : hs + DG, :].T, KD)
                    .reshape(P, KD, MQ, P)
                    .transpose(0, 2, 1, 3),
                    bf16,
                ),
                "wk": c(
                    kp(Wk[hs : hs + DG, :].T, KD)
                    .reshape(P, KD, MQ, P)
                    .transpose(0, 2, 1, 3),
                    bf16,
                ),
                "wv": c(kp(Wv[hs : hs + DG, :].T, KD), bf16),
                "wo": c(kp(Wo[:, hs : hs + DG].T, MQ), bf16),
                "bq": c(bq[hs : hs + DG].reshape(MQ, P).T, f32),
                "bk": c(bk[hs : hs + DG].reshape(MQ, P).T, f32),
            }
        )
    return in_maps


def _run(inputs, trace=False):
    from concourse import bass_utils

    nc = _get_nc()
    np_in = {k: np.asarray(v) for k, v in inputs.items()}
    in_maps = _shard_inputs(**np_in)
    res = bass_utils.run_bass_kernel_spmd(
        nc, in_maps, core_ids=list(range(NCORES)), trace=trace
    )
    # softmax rows sum to 1, so the V bias passes through attention as
    # exactly +bv; fold bv @ Wo.T (and bo) on the host.
    corr = np_in["bo"].astype(np.float32) + np_in["bv"].astype(
        np.float32
    ) @ np_in["Wo"].astype(np.float32).T
    y = np.empty((B, T, D), dtype=np.float32)
    for b in range(B):
        acc = res.results[2 * b]["yT"].astype(np.float32) + res.results[
            2 * b + 1
        ]["yT"].astype(np.float32)  # [P, D/P, T]
        y[b] = acc.transpose(1, 0, 2).reshape(D, T).T + corr
    return y, res


def kernel(**inputs):
    y, _ = _run(inputs)
    return y


# revision 49
# speedup vs baseline: 1.0007x; 1.0007x over previous
"""Multi-head attention (B=4, T=2048, D=1024, H=16) on 8 Trainium2 NeuronCores.

Sharding: core = (batch, head-group): b = core // 2, g = core % 2.
Each core computes heads [g*8, g*8+8) of batch b:
  - Q/K projections into transposed layout qT/kT = W_g @ x_b.T  [512, 2048]
  - V projection in natural layout [2048, 512] plus a ones column per head
  - scores computed transposed: S.T tile = K_h @ Q_h.T on the PE; exp fused
    on ScalarE over two-bank PSUM groups (FD=1024), scale=1/sqrt(64),
    no max subtraction (logits ~N(0,1))
  - PV transposed: out[128q, 65] = pT-slice[128k,128q].T @ [V_h|1][128k,65]
    so the matmul streams only 65 columns per k-tile (the PE charges
    N=out-free-size) and the softmax row-sum lands lane-wise in column 64
  - normalize is a per-partition reciprocal + tensor_scalar_mul on DVE
  - o is transposed back for the output projection: XBAR dma transpose when
    deferred (rows 0-2), PE transpose-matmul on the latency-critical last row
  - partial output projection yT_g = Wo[:, g].T-contraction  [1024, 2048] bf16
Host: y[b] = (yT_part[2b] + yT_part[2b+1]).T + bo + bv @ Wo.T
(softmax rows sum to one, so the V bias contributes exactly bv @ Wo.T).

Schedule: pair-row outer, query-chunk inner; 16 windows of 16 score/exp
groups each. ScalarE's exp stream (256 x ~1.04us) is the near-critical
engine, so every other PE obligation — the trailing PV of the previous
window, Q/K prefetch for the next pair-row, the V projection (spread one
head-pair per row across the row boundary), and the output projection of
the previous chunk — is emitted as generator "filler" steps pulled between
score groups at ~600 ns of filler per group, keeping both PE and ScalarE
continuously busy. Input DMAs are few and big (shared-HWDGE cost), ordered
exactly as the prologue consumes them, with mt-major weight layouts so the
first projection slices are single fat descriptors.

Self-contained: hardcodes all shapes; requires only concourse (bass) + numpy.
"""

import numpy as np

B, T, D = 4, 2048, 1024
H, HD = 16, 64
HG, DG = 8, 512          # heads / feature columns per core
NCORES = 8
P = 128
KD = D // P              # 8  k-tiles over model dim
MQ = DG // P             # 4  partition tiles of qT/kT/oT (one per head pair)
TK = T // P              # 16 key tiles
TQC = 512                # query-chunk (= one fp32 PSUM bank)
NC2 = T // TQC           # 4  query chunks
VW = HD + 1              # V columns per head incl. ones column
SCALE = 0.125            # 1/sqrt(HD)
PULL_RATE = 600.0        # ns of filler pulled per score group
DMA_TP = True            # o-transpose on the DMA XBAR instead of the PE

_CACHE: dict = {}


def _emit(tc, aps, reps=1):
    import concourse.bass as bass  # noqa: F401
    from concourse import masks, mybir

    nc = tc.nc
    dt = mybir.dt
    f32, bf16 = dt.float32, dt.bfloat16
    AF = mybir.ActivationFunctionType
    xT, wq, wk, wv, wo, bq, bk, yT = (
        aps["xT"], aps["wq"], aps["wk"], aps["wv"], aps["wo"],
        aps["bq"], aps["bk"], aps["yT"],
    )

    from contextlib import ExitStack

    with ExitStack() as ctx:
        const = ctx.enter_context(tc.tile_pool(name="const", bufs=1))
        persist = ctx.enter_context(tc.tile_pool(name="persist", bufs=1))
        xw = ctx.enter_context(tc.tile_pool(name="xw", bufs=1))
        ptp = ctx.enter_context(tc.tile_pool(name="ptp", bufs=4))
        onp = ctx.enter_context(tc.tile_pool(name="onp", bufs=3))
        yop = ctx.enter_context(tc.tile_pool(name="yop", bufs=3))
        nrm = ctx.enter_context(tc.tile_pool(name="nrm", bufs=3))
        scps = ctx.enter_context(tc.tile_pool(name="scps", bufs=2, space="PSUM"))
        qkvps = ctx.enter_context(tc.tile_pool(name="qkvps", bufs=2, space="PSUM"))
        pvtp = ctx.enter_context(tc.tile_pool(name="pvtp", bufs=2, space="PSUM"))

        # ---- persistent SBUF ----
        q_sb = persist.tile([P, MQ, T], bf16)
        k_sb = persist.tile([P, MQ, T], bf16)
        v_sb = persist.tile([P, TK, HG * VW], bf16)
        oT_sb = persist.tile([P, MQ, T], bf16)
        v4d = v_sb.rearrange("p t (h c) -> p t h c", h=HG)
        nc.vector.memset(v4d[:, :, :, HD : HD + 1], 1.0)
        ident = const.tile([P, P], bf16)
        masks.make_identity(nc, ident)

        # ---- input DMAs. The shared HWDGE device costs ~625 ns per DMA, so
        # transfers are few and big, emitted on one queue in exactly the
        # order the prologue consumes them: the mt0 slices of wk/wq and x
        # chunk 0 arrive first (split in ki halves so the first projection
        # groups overlap the transfer), then the later x chunks, then the
        # weights the fillers need later.
        x_sb = xw.tile([P, KD, T], bf16)
        wq_sb = xw.tile([P, MQ, KD, P], bf16)
        wk_sb = xw.tile([P, MQ, KD, P], bf16)
        bq_sb = const.tile([P, MQ], f32)
        bk_sb = const.tile([P, MQ], f32)
        wv_sb = xw.tile([P, KD, DG], bf16)
        wo_sb = const.tile([P, MQ, D], bf16)
        KH = KD // 2
        nc.sync.dma_start(out=wk_sb[:, 0], in_=wk[:, 0])
        nc.sync.dma_start(out=x_sb[:, 0:KH, 0:TQC], in_=xT[:, 0:KH, 0:TQC])
        nc.sync.dma_start(out=wq_sb[:, 0], in_=wq[:, 0])
        nc.sync.dma_start(out=x_sb[:, KH:KD, 0:TQC], in_=xT[:, KH:KD, 0:TQC])
        nc.sync.dma_start(out=bq_sb, in_=bq)
        nc.sync.dma_start(out=bk_sb, in_=bk)
        nc.sync.dma_start(out=wv_sb, in_=wv)
        for xc in (1, 2, 3):
            s = slice(xc * TQC, (xc + 1) * TQC)
            nc.sync.dma_start(out=x_sb[:, :, s], in_=xT[:, :, s])
        nc.sync.dma_start(out=wk_sb[:, 1:MQ], in_=wk[:, 1:MQ])
        nc.sync.dma_start(out=wq_sb[:, 1:MQ], in_=wq[:, 1:MQ])
        nc.sync.dma_start(out=wo_sb, in_=wo)

        # accumulation order matches the ki-half arrival of the first pieces
        ki_order = list(range(KD))

        def gen_qk(mt, n, which="kq"):
            """Generator: one T-chunk (n) of the q and/or k projection."""
            sel = {
                "k": ((wk_sb, bk_sb, k_sb),),
                "q": ((wq_sb, bq_sb, q_sb),),
                "kq": ((wk_sb, bk_sb, k_sb), (wq_sb, bq_sb, q_sb)),
            }[which]
            for w_sb, b_col, dst in sel:
                ps = qkvps.tile([P, TQC], f32, tag="qkv", name="ps_qkv")
                for idx, ki in enumerate(ki_order):
                    nc.tensor.matmul(
                        ps,
                        w_sb[:, mt, ki, :],
                        x_sb[:, ki, n * TQC : (n + 1) * TQC],
                        start=(idx == 0),
                        stop=(idx == KD - 1),
                    )
                    if idx % 2 == 1 and idx < KD - 1:
                        yield 427
                nc.vector.tensor_scalar_add(
                    dst[:, mt, n * TQC : (n + 1) * TQC], ps, b_col[:, mt : mt + 1]
                )
                yield 427

        def gen_v(vp, t0, t1):
            """Generator: V projection of head pair vp for token tiles
            [t0, t1) — split by pair so it spreads across rows 0-2."""
            for t in range(t0, t1):
                ps = qkvps.tile([P, P], f32, tag="qkv", name="ps_v")
                for idx, ki in enumerate(ki_order):
                    nc.tensor.matmul(
                        ps,
                        x_sb[:, ki, t * P : (t + 1) * P],
                        wv_sb[:, ki, vp * P : (vp + 1) * P],
                        start=(idx == 0),
                        stop=(idx == KD - 1),
                    )
                    if idx == 3:
                        yield 213
                nc.vector.tensor_copy(
                    v4d[:, t, 2 * vp : 2 * vp + 2, 0:HD],
                    ps.rearrange("p (h c) -> p h c", h=2),
                )
                yield 213

        def gen_pvt(p, tq0, qw, pts, holder=None):
            """Two generators (head 0, head 1) of transposed PV + normalize
            (+ o-transpose unless deferred via holder) for the window at
            (p, tq0, qw). Head 1 depends on the last exps of that window's
            stream, so its generator is scheduled at the end of the next
            window's fillers."""
            nqt = qw // P
            ntile = (nqt + 1) // 2
            st = {}

            def part(i):
                if i == 0:
                    st["o_nat"] = onp.tile([P, nqt, P], bf16, name="onat")
                    if holder is not None:
                        holder["o_nat"] = st["o_nat"]
                    st["pv"] = [
                        pvtp.tile([P, 2, 2, VW], f32, tag="pv", name=f"pv{t}")
                        for t in range(ntile)
                    ]
                o_nat, pv = st["o_nat"], st["pv"]
                h = 2 * p + i
                # a matmul start lazily zeroes its whole 2KB psum bank, so
                # each (t, j, i) accumulation group runs start-to-stop
                # before the next opens
                for t in range(ntile):
                    for j in range(2):
                        qs = (2 * t + j) * P
                        for half in range(2):
                            for tk in range(8 * half, 8 * half + 8):
                                nc.tensor.matmul(
                                    pv[t][:, j, i, :],
                                    pts[i][:, tk, qs : qs + P],
                                    v_sb[:, tk, h * VW : (h + 1) * VW],
                                    start=(tk == 0),
                                    stop=(tk == TK - 1),
                                )
                            yield 217
                if i == 0:
                    return
                for t in range(ntile):
                    rc = nrm.tile([P, 2, 2, 1], f32, name="rc")
                    nc.vector.reciprocal(rc, pv[t][:, :, :, HD : HD + 1])
                    for j in range(2):
                        for ii in range(2):
                            nc.vector.tensor_scalar_mul(
                                o_nat[:, 2 * t + j, ii * HD : (ii + 1) * HD],
                                pv[t][:, j, ii, 0:HD],
                                rc[:, j, ii, :],
                            )
                    yield 0
                if holder is None:
                    yield from gen_tp(p, tq0, qw, o_nat)

            return part(0), part(1)

        def gen_tp(p, tq0, qw, o_nat, dma=False):
            if isinstance(o_nat, dict):
                o_nat = o_nat["o_nat"]
            nqt = qw // P
            if DMA_TP and dma:
                # XBAR transpose: out[dg, qt, q] = o_nat[q, qt, dg]; no PE or
                # DVE time, just a DMA queue slot
                nc.sync.dma_start_transpose(
                    out=oT_sb[:, p, tq0 : tq0 + qw].rearrange(
                        "p (a b) -> p a b", a=nqt
                    ),
                    in_=o_nat[:, :, :],
                )
                yield 213
                return
            tpv = pvtp.tile([P, nqt, P], bf16, tag="pv", name="tpv")
            for qt in range(nqt):
                nc.tensor.transpose(tpv[:, qt, :], o_nat[:, qt, :], ident)
            nc.vector.tensor_copy(
                oT_sb[:, p, tq0 : tq0 + qw], tpv.rearrange("p a b -> p (a b)")
            )
            yield 213

        def gen_oproj(tq0, qw):
            for j in range(D // P):
                ys = qkvps.tile([P, qw], f32, tag="qkv", name="ys")
                for ki in range(MQ):
                    nc.tensor.matmul(
                        ys,
                        wo_sb[:, ki, j * P : (j + 1) * P],
                        oT_sb[:, ki, tq0 : tq0 + qw],
                        start=(ki == 0),
                        stop=(ki == MQ - 1),
                    )
                yo = yop.tile([P, qw], bf16, name="yo")
                nc.vector.tensor_copy(yo, ys)
                nc.sync.dma_start(out=yT[:, j, tq0 : tq0 + qw], in_=yo)
                yield 853 * qw // TQC

        def gen_spacer(ns):
            yield ns

        def sc_group(pt, p, hb, tq0, qw, g):
            scs = scps.tile([P, 2, qw], f32, tag="sc", name="sc")
            for u in range(2):
                tk = 2 * g + u
                nc.tensor.matmul(
                    scs[:, u, :],
                    k_sb[hb : hb + HD, p, tk * P : (tk + 1) * P],
                    q_sb[hb : hb + HD, p, tq0 : tq0 + qw],
                    start=True,
                    stop=True,
                )
            nc.scalar.activation(pt[:, 2 * g : 2 * g + 2, :], scs, AF.Exp, scale=SCALE)

        def drain(gens):
            while gens:
                try:
                    return next(gens[0])
                except StopIteration:
                    gens.pop(0)
            return None

        def window(p, tq0, qw, pts, gens, interleave_heads=False, skip=0):
            # head-major keeps each head's exps contiguous so the trailing PV
            # (head-outer) never waits; window (0,0) instead paces key-tiles
            # slowest (head-interleaved) so the k(mt0, n) fillers stay ahead
            # of the score groups that read them
            if interleave_heads:
                order = [(g // 2, g % 2) for g in range(16)]
            else:
                order = [(g % 8, g // 8) for g in range(16)][skip:]
            rate = PULL_RATE * qw / TQC
            debt = 0.0
            for g, i in order:
                sc_group(pts[i], p, i * HD, tq0, qw, g)
                debt += rate
                while debt > 0:
                    step = drain(gens)
                    if step is None:
                        debt = 0.0
                        break
                    debt -= max(step, 180)
            while drain(gens) is not None:
                pass

        # ---- schedule: pair-outer, chunk-inner; PV trails one window;
        # V-projection of pair p+1 spreads across row p to even out the
        # per-window PE load against ScalarE's fixed 16-exp stream ----
        if reps > 1:
            loop_cm = tc.For_i(0, reps, 1)
            loop_cm.__enter__()

        # interleave the first k/q half-groups with the split x chunk-0
        # arrival: k(ki 0-3) runs during wq0's transfer, q(ki 0-3) during
        # the second x half's
        gk, gq = gen_qk(0, 0, "k"), gen_qk(0, 0, "q")
        for g_ in (gk, gq, gk, gq):
            next(g_)
            next(g_, None)
        for g_ in (gk, gq):
            for _ in g_:
                pass

        prev = None
        tp_pending = []  # deferred o-transposes for rows 0-2
        full = [(n * TQC, TQC) for n in range(NC2)]
        last_row = full
        pts_next, banked = None, 0
        for p in range(MQ):
            chunks = last_row if p == MQ - 1 else full
            for c, (tq0, qw) in enumerate(chunks):
                if pts_next is not None:
                    pts, skip = pts_next, banked
                    pts_next, banked = None, 0
                else:
                    skip = 0
                    pts = [
                        ptp.tile([P, TK, qw], bf16, tag="pt", name="pt0"),
                        ptp.tile([P, TK, qw], bf16, tag="pt", name="pt1"),
                    ]
                gens = []
                if p == 0 and c == 0:
                    # k(mt0, n) fillers interleave with short V slices; the
                    # pull budget completes chunk n before the score groups
                    # that read it (emission-order margins checked)
                    gens += [gen_qk(0, 1, "k"), gen_v(0, 0, 2),
                             gen_qk(0, 2, "k"), gen_v(0, 2, 4),
                             gen_qk(0, 3, "k"), gen_v(0, 4, TK)]
                else:
                    # V of pair p+1 spreads over windows (p,1)..(p+1,0): the
                    # row-boundary windows are otherwise the lightest
                    vp, vt = (p + 1, 4 * (c - 1)) if c else (p, 12)
                    if vp <= MQ - 1:
                        gens.append(gen_v(vp, vt, vt + 4))
                if len(tp_pending) > 1 or (tp_pending and p == MQ - 1):
                    gens.append(gen_tp(*tp_pending.pop(0), dma=True))
                pvt1 = None
                if prev is not None:
                    pp, ptq0, pqw, ppts = prev
                    if pp == MQ - 1:
                        pvt0, pvt1 = gen_pvt(pp, ptq0, pqw, ppts)
                    else:
                        holder = {}
                        pvt0, pvt1 = gen_pvt(pp, ptq0, pqw, ppts, holder)
                        tp_pending.append((pp, ptq0, pqw, holder))
                    gens.append(pvt0)
                    gens.append(pvt1)
                    pvt1 = None
                if p == 0 and c < NC2 - 1:
                    gens.append(gen_qk(0, c + 1, "q"))
                if p < MQ - 1:
                    which = "k" if (p == MQ - 2 and c == NC2 - 1) else "kq"
                    gens.append(gen_qk(p + 1, c, which))
                if p == MQ - 1 and c == 0:
                    gens.append(gen_qk(MQ - 1, NC2 - 1, "q"))
                if pvt1 is not None:
                    gens.append(pvt1)
                if p == MQ - 1 and c > 0:
                    gens.append(gen_spacer(1200))
                    gens.append(gen_oproj(*chunks[c - 1]))
                window(p, tq0, qw, pts, gens,
                       interleave_heads=(p == 0 and c == 0), skip=skip)
                prev = (p, tq0, qw, pts)

        # epilogue: leftover transposes, last window's PV + final projection
        gens = [gen_tp(*t, dma=True) for t in tp_pending]
        gens += [*gen_pvt(*prev), gen_spacer(600), gen_oproj(*last_row[-1])]
        while drain(gens) is not None:
            pass

        if reps > 1:
            loop_cm.__exit__(None, None, None)


def _build(debug=False, reps=1):
    import concourse.tile as tile
    from concourse import bacc, mybir

    dt = mybir.dt
    f32, bf16 = dt.float32, dt.bfloat16

    nc = bacc.Bacc("TRN2", target_bir_lowering=False, debug=False)
    # inputs are host-preswizzled into partition-major layouts so every DMA
    # descriptor is a fat contiguous run
    aps = {
        "xT": nc.dram_tensor("xT", [P, KD, T], bf16, kind="ExternalInput").ap(),
        "wq": nc.dram_tensor(
            "wq", [P, MQ, KD, P], bf16, kind="ExternalInput"
        ).ap(),
        "wk": nc.dram_tensor(
            "wk", [P, MQ, KD, P], bf16, kind="ExternalInput"
        ).ap(),
        "wv": nc.dram_tensor("wv", [P, KD, DG], bf16, kind="ExternalInput").ap(),
        "wo": nc.dram_tensor("wo", [P, MQ, D], bf16, kind="ExternalInput").ap(),
        "bq": nc.dram_tensor("bq", [P, MQ], f32, kind="ExternalInput").ap(),
        "bk": nc.dram_tensor("bk", [P, MQ], f32, kind="ExternalInput").ap(),
        "yT": nc.dram_tensor("yT", [P, D // P, T], bf16, kind="ExternalOutput").ap(),
    }

    with tile.TileContext(nc) as tc:
        _emit(tc, aps, reps=reps)
    nc.compile()
    return nc


def _get_nc():
    if "nc" not in _CACHE:
        _CACHE["nc"] = _build()
    return _CACHE["nc"]


def _shard_inputs(x, Wq, bq, Wk, bk, Wv, bv, Wo, bo):
    import ml_dtypes

    bf16 = ml_dtypes.bfloat16
    f32 = np.float32

    def c(a, dtype):
        return np.ascontiguousarray(a).astype(dtype)

    def kp(a, kt):  # [kt*P, F] -> [P, kt, F] partition-major swizzle
        return a.reshape(kt, P, a.shape[-1]).transpose(1, 0, 2)

    in_maps = []
    for core in range(NCORES):
        b, g = core // 2, core % 2
        hs = g * DG
        in_maps.append(
            {
                "xT": c(kp(x[b].T, KD), bf16),
                "wq": c(
                    kp(Wq[hs : hs + DG, :].T, KD)
                    .reshape(P, KD, MQ, P)
                    .transpose(0, 2, 1, 3),
                    bf16,
                ),
                "wk": c(
                    kp(Wk[hs : hs + DG, :].T, KD)
                    .reshape(P, KD, MQ, P)
                    .transpose(0, 2, 1, 3),
                    bf16,
                ),
                "wv": c(kp(Wv[hs : hs + DG, :].T, KD), bf16),
                "wo": c(kp(Wo[:, hs : hs + DG].T, MQ), bf16),
                "bq": c(bq[hs : hs + DG].reshape(MQ, P).T, f32),
                "bk": c(bk[hs : hs + DG].reshape(MQ, P).T, f32),
            }
        )
    return in_maps


def _run(inputs, trace=False):
    from concourse import bass_utils

    nc = _get_nc()
    np_in = {k: np.asarray(v) for k, v in inputs.items()}
    in_maps = _shard_inputs(**np_in)
    res = bass_utils.run_bass_kernel_spmd(
        nc, in_maps, core_ids=list(range(NCORES)), trace=trace
    )
    # softmax rows sum to 1, so the V bias passes through attention as
    # exactly +bv; fold bv @ Wo.T (and bo) on the host.
    corr = np_in["bo"].astype(np.float32) + np_in["bv"].astype(
        np.float32
    ) @ np_in["Wo"].astype(np.float32).T
    y = np.empty((B, T, D), dtype=np.float32)
    for b in range(B):
        acc = res.results[2 * b]["yT"].astype(np.float32) + res.results[
            2 * b + 1
        ]["yT"].astype(np.float32)  # [P, D/P, T]
        y[b] = acc.transpose(1, 0, 2).reshape(D, T).T + corr
    return y, res


def kernel(**inputs):
    y, _ = _run(inputs)
    return y


# revision 52
# speedup vs baseline: 1.0010x; 1.0003x over previous
"""Multi-head attention (B=4, T=2048, D=1024, H=16) on 8 Trainium2 NeuronCores.

Sharding: core = (batch, head-group): b = core // 2, g = core % 2.
Each core computes heads [g*8, g*8+8) of batch b:
  - Q/K projections into transposed layout qT/kT = W_g @ x_b.T  [512, 2048]
  - V projection in natural layout [2048, 512] plus a ones column per head
  - scores computed transposed: S.T tile = K_h @ Q_h.T on the PE; exp fused
    on ScalarE over two-bank PSUM groups (FD=1024), scale=1/sqrt(64),
    no max subtraction (logits ~N(0,1))
  - PV transposed: out[128q, 65] = pT-slice[128k,128q].T @ [V_h|1][128k,65]
    so the matmul streams only 65 columns per k-tile (the PE charges
    N=out-free-size) and the softmax row-sum lands lane-wise in column 64
  - normalize is a per-partition reciprocal + tensor_scalar_mul on DVE
  - o is transposed back for the output projection: XBAR dma transpose when
    deferred (rows 0-2), PE transpose-matmul on the latency-critical last row
  - partial output projection yT_g = Wo[:, g].T-contraction  [1024, 2048] bf16
Host: y[b] = (yT_part[2b] + yT_part[2b+1]).T + bo + bv @ Wo.T
(softmax rows sum to one, so the V bias contributes exactly bv @ Wo.T).

Schedule: pair-row outer, query-chunk inner; 16 windows of 16 score/exp
groups each. ScalarE's exp stream (256 x ~1.04us) is the near-critical
engine, so every other PE obligation — the trailing PV of the previous
window, Q/K prefetch for the next pair-row, the V projection (spread one
head-pair per row across the row boundary), and the output projection of
the previous chunk — is emitted as generator "filler" steps pulled between
score groups at ~600 ns of filler per group, keeping both PE and ScalarE
continuously busy. Input DMAs are few and big (shared-HWDGE cost), ordered
exactly as the prologue consumes them, with mt-major weight layouts so the
first projection slices are single fat descriptors.

Self-contained: hardcodes all shapes; requires only concourse (bass) + numpy.
"""

import numpy as np

B, T, D = 4, 2048, 1024
H, HD = 16, 64
HG, DG = 8, 512          # heads / feature columns per core
NCORES = 8
P = 128
KD = D // P              # 8  k-tiles over model dim
MQ = DG // P             # 4  partition tiles of qT/kT/oT (one per head pair)
TK = T // P              # 16 key tiles
TQC = 512                # query-chunk (= one fp32 PSUM bank)
NC2 = T // TQC           # 4  query chunks
VW = HD + 1              # V columns per head incl. ones column
SCALE = 0.125            # 1/sqrt(HD)
PULL_RATE = 600.0        # ns of filler pulled per score group
DMA_TP = True            # o-transpose on the DMA XBAR instead of the PE

_CACHE: dict = {}


def _emit(tc, aps, reps=1):
    import concourse.bass as bass  # noqa: F401
    from concourse import masks, mybir

    nc = tc.nc
    dt = mybir.dt
    f32, bf16 = dt.float32, dt.bfloat16
    AF = mybir.ActivationFunctionType
    xT, wq, wk, wv, wo, bq, bk, yT = (
        aps["xT"], aps["wq"], aps["wk"], aps["wv"], aps["wo"],
        aps["bq"], aps["bk"], aps["yT"],
    )

    from contextlib import ExitStack

    with ExitStack() as ctx:
        const = ctx.enter_context(tc.tile_pool(name="const", bufs=1))
        persist = ctx.enter_context(tc.tile_pool(name="persist", bufs=1))
        xw = ctx.enter_context(tc.tile_pool(name="xw", bufs=1))
        ptp = ctx.enter_context(tc.tile_pool(name="ptp", bufs=4))
        onp = ctx.enter_context(tc.tile_pool(name="onp", bufs=3))
        yop = ctx.enter_context(tc.tile_pool(name="yop", bufs=3))
        nrm = ctx.enter_context(tc.tile_pool(name="nrm", bufs=3))
        scps = ctx.enter_context(tc.tile_pool(name="scps", bufs=2, space="PSUM"))
        qkvps = ctx.enter_context(tc.tile_pool(name="qkvps", bufs=2, space="PSUM"))
        pvtp = ctx.enter_context(tc.tile_pool(name="pvtp", bufs=2, space="PSUM"))

        # ---- persistent SBUF ----
        q_sb = persist.tile([P, MQ, T], bf16)
        k_sb = persist.tile([P, MQ, T], bf16)
        v_sb = persist.tile([P, TK, HG * VW], bf16)
        oT_sb = persist.tile([P, MQ, T], bf16)
        v4d = v_sb.rearrange("p t (h c) -> p t h c", h=HG)
        nc.vector.memset(v4d[:, :, :, HD : HD + 1], 1.0)
        ident = const.tile([P, P], bf16)
        masks.make_identity(nc, ident)

        # ---- input DMAs. The shared HWDGE device costs ~625 ns per DMA, so
        # transfers are few and big, emitted on one queue in exactly the
        # order the prologue consumes them: the mt0 slices of wk/wq and x
        # chunk 0 arrive first (split in ki halves so the first projection
        # groups overlap the transfer), then the later x chunks, then the
        # weights the fillers need later.
        x_sb = xw.tile([P, KD, T], bf16)
        wq_sb = xw.tile([P, MQ, KD, P], bf16)
        wk_sb = xw.tile([P, MQ, KD, P], bf16)
        bq_sb = const.tile([P, MQ], f32)
        bk_sb = const.tile([P, MQ], f32)
        wv_sb = xw.tile([P, KD, DG], bf16)
        wo_sb = const.tile([P, MQ, D], bf16)
        KH = KD // 2
        nc.sync.dma_start(out=wk_sb[:, 0], in_=wk[:, 0])
        nc.sync.dma_start(out=x_sb[:, 0:KH, 0:TQC], in_=xT[:, 0:KH, 0:TQC])
        nc.sync.dma_start(out=wq_sb[:, 0], in_=wq[:, 0])
        nc.sync.dma_start(out=x_sb[:, KH:KD, 0:TQC], in_=xT[:, KH:KD, 0:TQC])
        nc.sync.dma_start(out=bq_sb, in_=bq)
        nc.sync.dma_start(out=bk_sb, in_=bk)
        nc.sync.dma_start(out=wv_sb, in_=wv)
        for xc in (1, 2, 3):
            s = slice(xc * TQC, (xc + 1) * TQC)
            nc.sync.dma_start(out=x_sb[:, :, s], in_=xT[:, :, s])
        nc.sync.dma_start(out=wk_sb[:, 1:MQ], in_=wk[:, 1:MQ])
        nc.sync.dma_start(out=wq_sb[:, 1:MQ], in_=wq[:, 1:MQ])
        nc.sync.dma_start(out=wo_sb, in_=wo)

        # accumulation order matches the ki-half arrival of the first pieces
        ki_order = list(range(KD))

        def gen_qk(mt, n, which="kq"):
            """Generator: one T-chunk (n) of the q and/or k projection."""
            sel = {
                "k": ((wk_sb, bk_sb, k_sb),),
                "q": ((wq_sb, bq_sb, q_sb),),
                "kq": ((wk_sb, bk_sb, k_sb), (wq_sb, bq_sb, q_sb)),
            }[which]
            for w_sb, b_col, dst in sel:
                ps = qkvps.tile([P, TQC], f32, tag="qkv", name="ps_qkv")
                for idx, ki in enumerate(ki_order):
                    nc.tensor.matmul(
                        ps,
                        w_sb[:, mt, ki, :],
                        x_sb[:, ki, n * TQC : (n + 1) * TQC],
                        start=(idx == 0),
                        stop=(idx == KD - 1),
                    )
                    if idx % 2 == 1 and idx < KD - 1:
                        yield 427
                nc.vector.tensor_scalar_add(
                    dst[:, mt, n * TQC : (n + 1) * TQC], ps, b_col[:, mt : mt + 1]
                )
                yield 427

        def gen_v(vp, t0, t1):
            """Generator: V projection of head pair vp for token tiles
            [t0, t1) — split by pair so it spreads across rows 0-2."""
            for t in range(t0, t1):
                ps = qkvps.tile([P, P], f32, tag="qkv", name="ps_v")
                for idx, ki in enumerate(ki_order):
                    nc.tensor.matmul(
                        ps,
                        x_sb[:, ki, t * P : (t + 1) * P],
                        wv_sb[:, ki, vp * P : (vp + 1) * P],
                        start=(idx == 0),
                        stop=(idx == KD - 1),
                    )
                    if idx == 3:
                        yield 213
                nc.vector.tensor_copy(
                    v4d[:, t, 2 * vp : 2 * vp + 2, 0:HD],
                    ps.rearrange("p (h c) -> p h c", h=2),
                )
                yield 213

        def gen_pvt(p, tq0, qw, pts, holder=None):
            """Two generators (head 0, head 1) of transposed PV + normalize
            (+ o-transpose unless deferred via holder) for the window at
            (p, tq0, qw). Head 1 depends on the last exps of that window's
            stream, so its generator is scheduled at the end of the next
            window's fillers."""
            nqt = qw // P
            ntile = (nqt + 1) // 2
            st = {}

            def part(i):
                if i == 0:
                    st["o_nat"] = onp.tile([P, nqt, P], bf16, name="onat")
                    if holder is not None:
                        holder["o_nat"] = st["o_nat"]
                    st["pv"] = [
                        pvtp.tile([P, 2, 2, VW], f32, tag="pv", name=f"pv{t}")
                        for t in range(ntile)
                    ]
                o_nat, pv = st["o_nat"], st["pv"]
                h = 2 * p + i
                # a matmul start lazily zeroes its whole 2KB psum bank, so
                # each (t, j, i) accumulation group runs start-to-stop
                # before the next opens
                for t in range(ntile):
                    for j in range(2):
                        qs = (2 * t + j) * P
                        for half in range(2):
                            for tk in range(8 * half, 8 * half + 8):
                                nc.tensor.matmul(
                                    pv[t][:, j, i, :],
                                    pts[i][:, tk, qs : qs + P],
                                    v_sb[:, tk, h * VW : (h + 1) * VW],
                                    start=(tk == 0),
                                    stop=(tk == TK - 1),
                                )
                            yield 217
                if i == 0:
                    return
                for t in range(ntile):
                    rc = nrm.tile([P, 2, 2, 1], f32, name="rc")
                    nc.vector.reciprocal(rc, pv[t][:, :, :, HD : HD + 1])
                    for j in range(2):
                        for ii in range(2):
                            nc.vector.tensor_scalar_mul(
                                o_nat[:, 2 * t + j, ii * HD : (ii + 1) * HD],
                                pv[t][:, j, ii, 0:HD],
                                rc[:, j, ii, :],
                            )
                    yield 0
                if holder is None:
                    yield from gen_tp(p, tq0, qw, o_nat)

            return part(0), part(1)

        def gen_tp(p, tq0, qw, o_nat, dma=False):
            if isinstance(o_nat, dict):
                o_nat = o_nat["o_nat"]
            nqt = qw // P
            if DMA_TP and dma:
                # XBAR transpose: out[dg, qt, q] = o_nat[q, qt, dg]; no PE or
                # DVE time, just a DMA queue slot
                nc.sync.dma_start_transpose(
                    out=oT_sb[:, p, tq0 : tq0 + qw].rearrange(
                        "p (a b) -> p a b", a=nqt
                    ),
                    in_=o_nat[:, :, :],
                )
                yield 213
                return
            tpv = pvtp.tile([P, nqt, P], bf16, tag="pv", name="tpv")
            for qt in range(nqt):
                nc.tensor.transpose(tpv[:, qt, :], o_nat[:, qt, :], ident)
            nc.vector.tensor_copy(
                oT_sb[:, p, tq0 : tq0 + qw], tpv.rearrange("p a b -> p (a b)")
            )
            yield 213

        def gen_oproj(tq0, qw):
            for j in range(D // P):
                ys = qkvps.tile([P, qw], f32, tag="qkv", name="ys")
                for ki in range(MQ):
                    nc.tensor.matmul(
                        ys,
                        wo_sb[:, ki, j * P : (j + 1) * P],
                        oT_sb[:, ki, tq0 : tq0 + qw],
                        start=(ki == 0),
                        stop=(ki == MQ - 1),
                    )
                yo = yop.tile([P, qw], bf16, name="yo")
                nc.vector.tensor_copy(yo, ys)
                nc.sync.dma_start(out=yT[:, j, tq0 : tq0 + qw], in_=yo)
                yield 853 * qw // TQC

        def gen_spacer(ns):
            yield ns

        def sc_group(pt, p, hb, tq0, qw, g):
            scs = scps.tile([P, 2, qw], f32, tag="sc", name="sc")
            for u in range(2):
                tk = 2 * g + u
                nc.tensor.matmul(
                    scs[:, u, :],
                    k_sb[hb : hb + HD, p, tk * P : (tk + 1) * P],
                    q_sb[hb : hb + HD, p, tq0 : tq0 + qw],
                    start=True,
                    stop=True,
                )
            nc.scalar.activation(pt[:, 2 * g : 2 * g + 2, :], scs, AF.Exp, scale=SCALE)

        def drain(gens):
            while gens:
                try:
                    return next(gens[0])
                except StopIteration:
                    gens.pop(0)
            return None

        def window(p, tq0, qw, pts, gens, interleave_heads=False, skip=0,
                   rate_mult=1.0):
            # head-major keeps each head's exps contiguous so the trailing PV
            # (head-outer) never waits; window (0,0) instead paces key-tiles
            # slowest (head-interleaved) so the k(mt0, n) fillers stay ahead
            # of the score groups that read them
            if interleave_heads:
                order = [(g // 2, g % 2) for g in range(16)]
            else:
                order = [(g % 8, g // 8) for g in range(16)][skip:]
            rate = PULL_RATE * qw / TQC * rate_mult
            debt = 0.0
            for g, i in order:
                sc_group(pts[i], p, i * HD, tq0, qw, g)
                debt += rate
                while debt > 0:
                    step = drain(gens)
                    if step is None:
                        debt = 0.0
                        break
                    debt -= max(step, 180)
            while drain(gens) is not None:
                pass

        # ---- schedule: pair-outer, chunk-inner; PV trails one window;
        # V-projection of pair p+1 spreads across row p to even out the
        # per-window PE load against ScalarE's fixed 16-exp stream ----
        if reps > 1:
            loop_cm = tc.For_i(0, reps, 1)
            loop_cm.__enter__()

        # interleave the first k/q half-groups with the split x chunk-0
        # arrival: k(ki 0-3) runs during wq0's transfer, q(ki 0-3) during
        # the second x half's
        gk, gq = gen_qk(0, 0, "k"), gen_qk(0, 0, "q")
        for g_ in (gk, gq, gk, gq):
            next(g_)
            next(g_, None)
        for g_ in (gk, gq):
            for _ in g_:
                pass

        prev = None
        tp_pending = []  # deferred o-transposes for rows 0-2
        full = [(n * TQC, TQC) for n in range(NC2)]
        last_row = full
        pts_next, banked = None, 0
        for p in range(MQ):
            chunks = last_row if p == MQ - 1 else full
            for c, (tq0, qw) in enumerate(chunks):
                if pts_next is not None:
                    pts, skip = pts_next, banked
                    pts_next, banked = None, 0
                else:
                    skip = 0
                    pts = [
                        ptp.tile([P, TK, qw], bf16, tag="pt", name="pt0"),
                        ptp.tile([P, TK, qw], bf16, tag="pt", name="pt1"),
                    ]
                gens = []
                if p == 0 and c == 0:
                    # k(mt0, n) fillers interleave with short V slices; the
                    # pull budget completes chunk n before the score groups
                    # that read it (emission-order margins checked)
                    gens += [gen_qk(0, 1, "k"), gen_v(0, 0, 2),
                             gen_qk(0, 2, "k"), gen_v(0, 2, 4),
                             gen_qk(0, 3, "k"), gen_v(0, 4, TK)]
                else:
                    # V of pair p+1 spreads over windows (p,1)..(p+1,0): the
                    # row-boundary windows are otherwise the lightest
                    vp, vt = (p + 1, 4 * (c - 1)) if c else (p, 12)
                    if vp <= MQ - 1:
                        gens.append(gen_v(vp, vt, vt + 4))
                if len(tp_pending) > 1 or (tp_pending and p == MQ - 1):
                    gens.append(gen_tp(*tp_pending.pop(0), dma=True))
                pvt1 = None
                if prev is not None:
                    pp, ptq0, pqw, ppts = prev
                    if pp == MQ - 1:
                        pvt0, pvt1 = gen_pvt(pp, ptq0, pqw, ppts)
                    else:
                        holder = {}
                        pvt0, pvt1 = gen_pvt(pp, ptq0, pqw, ppts, holder)
                        tp_pending.append((pp, ptq0, pqw, holder))
                    gens.append(pvt0)
                    gens.append(pvt1)
                    pvt1 = None
                if p == 0 and c < NC2 - 1:
                    gens.append(gen_qk(0, c + 1, "q"))
                if p < MQ - 1:
                    which = "k" if (p == MQ - 2 and c == NC2 - 1) else "kq"
                    gens.append(gen_qk(p + 1, c, which))
                if p == MQ - 1 and c == 0:
                    gens.append(gen_qk(MQ - 1, NC2 - 1, "q"))
                if pvt1 is not None:
                    gens.append(pvt1)
                if p == MQ - 1 and c > 0:
                    gens.append(gen_spacer(1200))
                    gens.append(gen_oproj(*chunks[c - 1]))
                window(p, tq0, qw, pts, gens,
                       interleave_heads=(p == 0 and c == 0), skip=skip,
                       rate_mult=(1.5 if (p, c) == (MQ - 1, NC2 - 1) else 1.0))
                prev = (p, tq0, qw, pts)

        # epilogue: leftover transposes, last window's PV + final projection
        gens = [gen_tp(*t, dma=True) for t in tp_pending]
        gens += [*gen_pvt(*prev), gen_spacer(600), gen_oproj(*last_row[-1])]
        while drain(gens) is not None:
            pass

        if reps > 1:
            loop_cm.__exit__(None, None, None)


def _build(debug=False, reps=1):
    import concourse.tile as tile
    from concourse import bacc, mybir

    dt = mybir.dt
    f32, bf16 = dt.float32, dt.bfloat16

    nc = bacc.Bacc("TRN2", target_bir_lowering=False, debug=False)
    # inputs are host-preswizzled into partition-major layouts so every DMA
    # descriptor is a fat contiguous run
    aps = {
        "xT": nc.dram_tensor("xT", [P, KD, T], bf16, kind="ExternalInput").ap(),
        "wq": nc.dram_tensor(
            "wq", [P, MQ, KD, P], bf16, kind="ExternalInput"
        ).ap(),
        "wk": nc.dram_tensor(
            "wk", [P, MQ, KD, P], bf16, kind="ExternalInput"
        ).ap(),
        "wv": nc.dram_tensor("wv", [P, KD, DG], bf16, kind="ExternalInput").ap(),
        "wo": nc.dram_tensor("wo", [P, MQ, D], bf16, kind="ExternalInput").ap(),
        "bq": nc.dram_tensor("bq", [P, MQ], f32, kind="ExternalInput").ap(),
        "bk": nc.dram_tensor("bk", [P, MQ], f32, kind="ExternalInput").ap(),
        "yT": nc.dram_tensor("yT", [P, D // P, T], bf16, kind="ExternalOutput").ap(),
    }

    with tile.TileContext(nc) as tc:
        _emit(tc, aps, reps=reps)
    nc.compile()
    return nc


def _get_nc():
    if "nc" not in _CACHE:
        _CACHE["nc"] = _build()
    return _CACHE["nc"]


def _shard_inputs(x, Wq, bq, Wk, bk, Wv, bv, Wo, bo):
    import ml_dtypes

    bf16 = ml_dtypes.bfloat16
    f32 = np.float32

    def c(a, dtype):
        return np.ascontiguousarray(a).astype(dtype)

    def kp(a, kt):  # [kt*P, F] -> [P, kt, F] partition-major swizzle
        return a.reshape(kt, P, a.shape[-1]).transpose(1, 0, 2)

    in_maps = []
    for core in range(NCORES):
        b, g = core // 2, core % 2
        hs = g * DG
        in_maps.append(
            {
                "xT": c(kp(x[b].T, KD), bf16),
                "wq": c(
                    kp(Wq[hs : hs + DG, :].T, KD)
                    .reshape(P, KD, MQ, P)
                    .transpose(0, 2, 1, 3),
                    bf16,
                ),
                "wk": c(
                    kp(Wk[hs : hs + DG, :].T, KD)
                    .reshape(P, KD, MQ, P)
                    .transpose(0, 2, 1, 3),
                    bf16,
                ),
                "wv": c(kp(Wv[hs : hs + DG, :].T, KD), bf16),
                "wo": c(kp(Wo[:, hs : hs + DG].T, MQ), bf16),
                "bq": c(bq[hs : hs + DG].reshape(MQ, P).T, f32),
                "bk": c(bk[hs : hs + DG].reshape(MQ, P).T, f32),
            }
        )
    return in_maps


def _run(inputs, trace=False):
    from concourse import bass_utils

    nc = _get_nc()
    np_in = {k: np.asarray(v) for k, v in inputs.items()}
    in_maps = _shard_inputs(**np_in)
    res = bass_utils.run_bass_kernel_spmd(
        nc, in_maps, core_ids=list(range(NCORES)), trace=trace
    )
    # softmax rows sum to 1, so the V bias passes through attention as
    # exactly +bv; fold bv @ Wo.T (and bo) on the host.
    corr = np_in["bo"].astype(np.float32) + np_in["bv"].astype(
        np.float32
    ) @ np_in["Wo"].astype(np.float32).T
    y = np.empty((B, T, D), dtype=np.float32)
    for b in range(B):
        acc = res.results[2 * b]["yT"].astype(np.float32) + res.results[
            2 * b + 1
        ]["yT"].astype(np.float32)  # [P, D/P, T]
        y[b] = acc.transpose(1, 0, 2).reshape(D, T).T + corr
    return y, res


def kernel(**inputs):
    y, _ = _run(inputs)
    return y
